# revision 34
# baseline (speedup 1.0000x reference)
"""Causal multi-head attention block (B=4,S=2048,E=1024,H=16,D=64) on 8 trn2 cores.

Sharding: 4 batches x 2 head-groups (8 heads each) = 8 cores.
Each core: QKV projection for its (batch, head-group), causal attention,
partial output projection over its heads. Host sums the 2 partials per batch
(the "all-reduce after project_out" done at gather time) and adds b_out.

Layout: everything is computed transposed; no on-chip transposes anywhere.
  qkv^T[f, s] = W^T x^T   via matmul(lhsT=W[e,f], rhs=xT[e,s])
  V natural [s, f]        via matmul(lhsT=xT[e,s], rhs=Wv[e,f])
  scores^T[k, q] = K Q^T  via matmul(lhsT=KT[d,k], rhs=QT[d,q]) per head (d=64);
                          head pairs use partition bases 0/64 -> concurrent
                          row-group matmuls on the PE array.
  softmax over k (= partition dim): exp on ACT (scale=1/sqrt(D) fused), the
  denominator comes free from a ones-column appended to V in the AV matmul,
  divide via DVE reciprocal + GpSimd partition_broadcast.
  ans^T[d, q]             via matmul(lhsT=[V|1][k, d+1], rhs=w^T[k, q])
  out^T[e, q] partial     via matmul(lhsT=Wout[f,e], rhs=ansT[f,q])

Causality: k-tiles above the diagonal are skipped outright; diagonal-band
tiles use partial-width matmuls/exp (columns >= j*128 only) plus a single
[128,128] triangle mask.

Scheduling: the attention inner loop is ACT(exp)-bound while projections are
pure PE work, so projection of s-block sb+1 and output-projection of q-block
qb-1 are emitted interleaved (generator round-robin) into attention(qb=sb)'s
instruction stream — the in-order PE engine then fills exp-latency with
projection matmuls. Matmuls run in float32r (full-rate PE mode, fp32 storage
with reduced-precision multiply, ~1e-4 relative error).
"""

import numpy as np

B, S, E, H, D = 4, 2048, 1024, 16, 64
NCORES = 8
HG = 2                 # head groups (tensor parallel)
HC = H // HG           # 8 heads per core
FQ = HC * D            # 512 local features per q/k/v
P, NB = 128, 512       # partition tile, free-dim block
ET, ST, KTN, FT = E // P, S // NB, S // P, FQ // P   # 8, 4, 16, 4

_cache = {}


def _build():
    from contextlib import ExitStack
    import concourse.tile as tile
    import concourse.mybir as mybir
    from concourse import bacc

    dt = mybir.dt
    f32, f32r = dt.float32, dt.float32r
    AF = mybir.ActivationFunctionType
    ALU = mybir.AluOpType
    SCALE = 0.125  # 1/sqrt(D)

    nc = bacc.Bacc("TRN2", target_bir_lowering=False, debug=False,
                   num_devices=NCORES)

    xT = nc.dram_tensor("xT", [E, S], f32r, kind="ExternalInput").ap()
    wq = nc.dram_tensor("wq", [E, FQ], f32r, kind="ExternalInput").ap()
    wk = nc.dram_tensor("wk", [E, FQ], f32r, kind="ExternalInput").ap()
    wv = nc.dram_tensor("wv", [E, FQ], f32r, kind="ExternalInput").ap()
    wo = nc.dram_tensor("wo", [FQ, E], f32r, kind="ExternalInput").ap()
    msk = nc.dram_tensor("msk", [P, P], f32, kind="ExternalInput").ap()
    bq = nc.dram_tensor("bq", [FQ], f32, kind="ExternalInput").ap()
    bk = nc.dram_tensor("bk", [FQ], f32, kind="ExternalInput").ap()
    bvb = nc.dram_tensor("bvb", [P, FQ], f32, kind="ExternalInput").ap()
    outT = nc.dram_tensor("outT", [E, S], f32, kind="ExternalOutput").ap()

    with tile.TileContext(nc) as tc:
        with ExitStack() as ctx:
            pers = ctx.enter_context(tc.tile_pool(name="pers", bufs=1))
            pmisc = ctx.enter_context(tc.tile_pool(name="pmisc", bufs=1))
            px = ctx.enter_context(tc.tile_pool(name="px", bufs=1))
            pw = ctx.enter_context(tc.tile_pool(name="pw", bufs=1))
            pqts = ctx.enter_context(tc.tile_pool(name="pqts", bufs=2))
            pwe = ctx.enter_context(tc.tile_pool(name="pwe", bufs=6))
            pans = ctx.enter_context(tc.tile_pool(name="pans", bufs=3))
            pepi = ctx.enter_context(tc.tile_pool(name="pepi", bufs=3))
            pout = ctx.enter_context(tc.tile_pool(name="pout", bufs=2))
            ps1 = ctx.enter_context(
                tc.tile_pool(name="ps1", bufs=2, space="PSUM"))
            sps = ctx.enter_context(
                tc.tile_pool(name="sps", bufs=2, space="PSUM"))
            avps = ctx.enter_context(
                tc.tile_pool(name="avps", bufs=2, space="PSUM"))

            KT = [pers.tile([P, S], f32r, tag=f"kt{i}", name=f"kt{i}")
                  for i in range(FT)]
            Vp = [pers.tile([P, HC * (D + 1)], f32r, tag=f"vp{i}",
                            name=f"vp{i}") for i in range(KTN)]
            bqt = pers.tile([P, FT], f32, tag="bqt")
            bkt = pers.tile([P, FT], f32, tag="bkt")
            bvt = pers.tile([P, FQ], f32, tag="bvt")
            onesf = pers.tile([P, HC], f32, tag="onesf")
            nc.vector.memset(onesf[:], 1.0)
            nc.sync.dma_start(bqt[:], bq.rearrange("(a p) -> p a", p=P))
            nc.sync.dma_start(bkt[:], bk.rearrange("(a p) -> p a", p=P))

            mtri = pmisc.tile([P, P], f32, tag="mtri")
            wouts = [pmisc.tile([P, E], f32r, tag=f"wo{ft}", name=f"wo{ft}")
                     for ft in range(FT)]

            def late_loads_gen():
                """Non-critical loads, emitted after proj(0)'s x/wq DMAs so
                they don't delay the first matmuls."""
                nc.scalar.dma_start(bvt[:], bvb[:])
                nc.scalar.dma_start(mtri[:], msk[:])
                yield

            def wout_gen():
                """wout loads; needed only by OUT(0), driven as a qb=0
                filler so they stay off the startup critical path."""
                for ft in range(FT):
                    eng = nc.scalar if ft % 2 else nc.sync
                    eng.dma_start(wouts[ft][:],
                                  wo[ft * P:(ft + 1) * P, :])
                    yield

            # per-block state shared between generators
            QTS = {}    # sb -> [4 tiles]
            ATS = {}    # qb -> [4 tiles]

            def proj_gen(sb):
                """QKV projection of s-block sb. Yields between PE chunks."""
                xts, wts = [], []
                for e in range(ET):
                    t = px.tile([P, NB], f32r, tag=f"x{e}", name=f"x{e}_{sb}")
                    nc.sync.dma_start(
                        t[:], xT[e * P:(e + 1) * P, sb * NB:(sb + 1) * NB])
                    xts.append(t)
                    t = pw.tile([P, FQ], f32r, tag=f"w{e}", name=f"wq{e}_{sb}")
                    # at startup both hwdge queues are idle; split the
                    # critical first x/wq loads across them
                    (nc.scalar if sb == 0 else nc.sync).dma_start(
                        t[:], wq[e * P:(e + 1) * P, :])
                    wts.append(t)
                yield
                QTS[sb] = []
                for ft in range(FT):
                    ps = ps1.tile([P, NB], f32, tag="ps", name=f"psq{ft}_{sb}")
                    for e in range(ET):
                        nc.tensor.matmul(ps[:], wts[e][:, ft * P:(ft + 1) * P],
                                         xts[e][:], start=(e == 0),
                                         stop=(e == ET - 1))
                        if e == 3:
                            yield
                    qt = pqts.tile([P, NB], f32r, tag=f"qts{ft}",
                                   name=f"qts{ft}_{sb}")
                    nc.vector.tensor_scalar_add(qt[:], ps[:],
                                                bqt[:, ft:ft + 1])
                    QTS[sb].append(qt)
                    yield
                # K pass
                wts = []
                for e in range(ET):
                    t = pw.tile([P, FQ], f32r, tag=f"w{e}", name=f"wk{e}_{sb}")
                    (nc.scalar if sb == 0 else nc.sync).dma_start(
                        t[:], wk[e * P:(e + 1) * P, :])
                    wts.append(t)
                yield
                for ft in range(FT):
                    ps = ps1.tile([P, NB], f32, tag="ps", name=f"psk{ft}_{sb}")
                    for e in range(ET):
                        nc.tensor.matmul(ps[:], wts[e][:, ft * P:(ft + 1) * P],
                                         xts[e][:], start=(e == 0),
                                         stop=(e == ET - 1))
                        if e == 3:
                            yield
                    nc.vector.tensor_scalar_add(
                        KT[ft][:, sb * NB:(sb + 1) * NB], ps[:],
                        bkt[:, ft:ft + 1])
                    yield
                # V pass
                wts = []
                for e in range(ET):
                    t = pw.tile([P, FQ], f32r, tag=f"w{e}", name=f"wv{e}_{sb}")
                    nc.sync.dma_start(t[:], wv[e * P:(e + 1) * P, :])
                    wts.append(t)
                yield
                for stl in range(ST):
                    st = ST * sb + stl
                    ps = ps1.tile([P, NB], f32, tag="ps",
                                  name=f"psv{stl}_{sb}")
                    for e in range(ET):
                        nc.tensor.matmul(ps[:],
                                         xts[e][:, stl * P:(stl + 1) * P],
                                         wts[e][:], start=(e == 0),
                                         stop=(e == ET - 1))
                        if e == 3:
                            yield
                    vview = Vp[st][:].rearrange("p (h c) -> p h c", c=D + 1)
                    nc.vector.tensor_copy(
                        vview[:, :, D:D + 1],
                        onesf[:].rearrange("p (h c) -> p h c", c=1))
                    nc.vector.scalar_tensor_tensor(
                        vview[:, :, 0:D], ps[:], 1.0,
                        bvt[:].rearrange("p (h d) -> p h d", d=D),
                        op0=ALU.mult, op1=ALU.add)
                    yield

            def attn_gen(qb):
                """Attention for q-block qb. Yields once per kt step."""
                nkt = ST * (qb + 1)
                QTs = QTS[qb]
                ATS[qb] = []
                for hp in range(FT):
                    at = pans.tile([P, NB], f32r, tag=f"at{hp}",
                                   name=f"at{hp}_{qb}")
                    ATS[qb].append(at)
                    av = [avps.tile([D + 1, NB], f32, tag="av",
                                    name=f"av{qb}_{hp}_{i}")
                          for i in range(2)]
                    def emit_av(ent, last):
                        k0, pc0, w0 = ent
                        for i in range(2):
                            nc.tensor.matmul(
                                av[i][:, pc0:NB],
                                Vp[k0][:, (2 * hp + i) * (D + 1):
                                                (2 * hp + i + 1) * (D + 1)],
                                w0[:, i * NB + pc0:(i + 1) * NB],
                                start=(k0 == 0), stop=last)

                    pend = []
                    for kt in range(nkt):
                        j = kt - ST * qb
                        c0 = j * P if j >= 0 else 0
                        # both heads of the pair share one 2-bank psum tile
                        # and a single strided exp call
                        sp = sps.tile([P, 2 * NB], f32, tag="sp",
                                      name=f"sp{qb}_{hp}_{kt}")
                        for i in range(2):
                            nc.tensor.matmul(
                                sp[:, i * NB + c0:(i + 1) * NB],
                                KT[hp][i * D:(i + 1) * D,
                                       kt * P:(kt + 1) * P],
                                QTs[hp][i * D:(i + 1) * D, c0:NB],
                                start=True, stop=True)
                        w = pwe.tile([P, 2 * NB], f32r, tag="w",
                                     name=f"w{qb}_{hp}_{kt}")
                        spv = sp[:].rearrange("p (h q) -> p h q", h=2)
                        wv_ = w[:].rearrange("p (h q) -> p h q", h=2)
                        nc.scalar.activation(wv_[:, :, c0:NB],
                                             spv[:, :, c0:NB],
                                             AF.Exp, scale=SCALE)
                        if j >= 0:
                            nc.vector.tensor_mul(
                                wv_[:, :, c0:c0 + P], wv_[:, :, c0:c0 + P],
                                mtri[:].rearrange("p (a q) -> p a q", a=1)
                                .broadcast_to([P, 2, P]))
                        pend.append((kt, c0, w))
                        if len(pend) > 2:
                            emit_av(pend.pop(0), last=False)
                        yield
                    while pend:
                        ent = pend.pop(0)
                        emit_av(ent, last=not pend)
                        yield
                    # epilogue: ats[hp][i*64:(i+1)*64, q] = av_i[d, q]/sum[q]
                    # raw av is copied out first so the psum slot frees for
                    # the next head pair; the divide happens in place on at.
                    # For the final pair there is no next pair -- mul straight
                    # from psum to shorten the chain into OUT(last).
                    last_pair = (qb == ST - 1 and hp == FT - 1)
                    for i in range(2):
                        se = pepi.tile([1, NB], f32, tag="se",
                                       name=f"se{qb}_{hp}_{i}")
                        nc.vector.tensor_copy(se[:], av[i][D:D + 1, :])
                        if not last_pair:
                            nc.vector.tensor_copy(at[i * D:(i + 1) * D, :],
                                                  av[i][0:D, :])
                        nc.vector.reciprocal_approx_fast(se[:], se[:])
                        bch = pepi.tile([P, NB], f32, tag="bch",
                                        name=f"bch{qb}_{hp}_{i}")
                        nc.gpsimd.partition_broadcast(
                            bch[0:(i + 1) * D, :], se[:],
                            channels=(i + 1) * D)
                        if last_pair:
                            nc.vector.tensor_mul(at[i * D:(i + 1) * D, :],
                                                 av[i][0:D, :],
                                                 bch[i * D:(i + 1) * D, :])
                        else:
                            nc.vector.tensor_mul(at[i * D:(i + 1) * D, :],
                                                 at[i * D:(i + 1) * D, :],
                                                 bch[i * D:(i + 1) * D, :])
                        yield

            def out_gen(qb):
                """Output projection of q-block qb. Yields per e-tile."""
                ats = ATS[qb]
                for et in range(ET):
                    po = ps1.tile([P, NB], f32, tag="ps", name=f"po{qb}_{et}")
                    for ft in range(FT):
                        nc.tensor.matmul(po[:],
                                         wouts[ft][:, et * P:(et + 1) * P],
                                         ats[ft][:], start=(ft == 0),
                                         stop=(ft == FT - 1))
                    ot = pout.tile([P, NB], f32, tag="ot",
                                   name=f"ot{qb}_{et}")
                    if qb == ST - 1:
                        # final q-block: ACT is idle by then, DVE is not
                        nc.scalar.copy(ot[:], po[:])
                    else:
                        nc.vector.tensor_copy(ot[:], po[:])
                    nc.gpsimd.dma_start(
                        outT[et * P:(et + 1) * P, qb * NB:(qb + 1) * NB],
                        ot[:])
                    yield

            def drain(g):
                for _ in g:
                    pass

            p0 = proj_gen(0)
            next(p0)          # x/wq DMAs emitted first
            drain(late_loads_gen())
            drain(p0)
            # Filler plan: spread PE-only work uniformly over each
            # attention block; OUT(1)/OUT(2) are deferred into attention(3),
            # which otherwise has no projection work left to hide exp latency.
            plans = {
                0: ([lambda: wout_gen(), lambda: proj_gen(1)], 43),
                1: ([lambda: proj_gen(2), lambda: out_gen(0)], 47),
                2: ([lambda: proj_gen(3)], 39),
                3: ([lambda: out_gen(1), lambda: out_gen(2)], 16),
            }
            for qb in range(ST):
                mk, nf = plans[qb]
                fillers = [m() for m in mk]
                na = 4 * (4 * (qb + 1) + 4)
                rate = nf / na
                acc, fi = 0.0, 0
                for _ in attn_gen(qb):
                    acc += rate
                    while acc >= 1.0 and fillers:
                        acc -= 1.0
                        f = fillers[fi % len(fillers)]
                        fi += 1
                        try:
                            next(f)
                        except StopIteration:
                            fillers.remove(f)
                for f in fillers:
                    drain(f)
            drain(out_gen(ST - 1))
    nc.compile()
    return nc


def _mask_tri():
    kp = np.arange(P)[:, None]
    qf = np.arange(P)[None, :]
    return (qf >= kp).astype(np.float32)


def kernel(x, W_qkv, b_qkv, W_out, b_out):
    from concourse.bass_utils import run_bass_kernel_spmd

    if "nc" not in _cache:
        _cache["nc"] = _build()
    nc = _cache["nc"]

    x = np.asarray(x, dtype=np.float32)
    W_qkv = np.asarray(W_qkv, dtype=np.float32)
    b_qkv = np.asarray(b_qkv, dtype=np.float32)
    W_out = np.asarray(W_out, dtype=np.float32)
    b_out = np.asarray(b_out, dtype=np.float32)

    mtri = _mask_tri()
    in_maps = []
    for c in range(NCORES):
        b, g = c % B, c // B
        hs = slice(g * HC, (g + 1) * HC)
        Wl = W_qkv[:, :, hs, :]                       # [E, 3, HC, D]
        in_maps.append({
            "xT": np.ascontiguousarray(x[b].T),
            "wq": np.ascontiguousarray(Wl[:, 0].reshape(E, FQ)),
            "wk": np.ascontiguousarray(Wl[:, 1].reshape(E, FQ)),
            "wv": np.ascontiguousarray(Wl[:, 2].reshape(E, FQ)),
            "wo": np.ascontiguousarray(W_out[hs].reshape(FQ, E)),
            "msk": mtri,
            "bq": np.ascontiguousarray(b_qkv[0, hs].reshape(FQ)),
            "bk": np.ascontiguousarray(b_qkv[1, hs].reshape(FQ)),
            "bvb": np.broadcast_to(b_qkv[2, hs].reshape(1, FQ),
                                   (P, FQ)).copy(),
        })

    res = run_bass_kernel_spmd(nc, in_maps, core_ids=list(range(NCORES)))
    _cache["last_results"] = res
    out = np.empty((B, S, E), dtype=np.float32)
    for b in range(B):
        out[b] = (res.results[b]["outT"].T + res.results[b + B]["outT"].T
                  + b_out)
    return out
